# Initial kernel scaffold
#
"""Trainium2 Bass kernel for nn_Decoder_10110353014984.

Computation (see reference): hard-reset LIF over T=4 steps followed by a
linear head:
    v' = v + (x_t - v)/2 ; spike = (v' >= 1) ; v = (1-spike) * v'
    y  = einsum('tbnd,cd->tbnc', spikes, W) + b

The LIF is replicated with the reference's exact fp32 rounding order:
    d = x - v ; h = 0.5*d (exact) ; v' = v + h ; spike = v' >= 1 ;
    v = v' * (v' < 1)

Sharding: data-parallel over batch B=64 -> 8 per NeuronCore. The host
pre-transposes each shard to xT[T, D, S] (d-major) so LIF spike tiles are
directly the matmul stationary operand (no on-chip transposes), and
pre-transposes W to W^T[D, C] for the moving operand.

Matmul dtype: float32r (TF32-like, 1s/8e/11m, ~4x the fp32 matmul rate).
Spikes are {0,1} (exact in fp32r). Default: W rounded to fp32r on the host,
single pass -> rel err ~1.2e-4 (TF32-class), ~130us/run. KERNEL_HILO=1
splits W into fp32r-exact hi+lo parts (hi+lo == W exactly: 12+12 mantissa
bits) accumulated in one PSUM group -> fp32-exact result (~2e-7) at ~1.6x
the time. Bias is applied host-side (zeros in the spec).
"""

import os
import sys
import types

sys.path.insert(0, "/opt/trn_rl_repo")

import numpy as np

import concourse.bass as bass
import concourse.mybir as mybir
import concourse.tile as tile
from concourse.vector_clock import ScopedClock
import bass_rust as _br

T, B, N, D, C = 4, 64, 196, 512, 1000
NCORES = 8
BL = B // NCORES          # 8 batches per core
S = BL * N                # 1568 samples per timestep per core
P = 128                   # partition width
DCH = D // P              # 4 contraction tiles
SCH = (S + P - 1) // P    # 13 sample chunks (last has 32 rows)
CHALF = [(0, 500), (500, 500)]  # C split across two PSUM banks

F32 = mybir.dt.float32
F32R = mybir.dt.float32r
ALU = mybir.AluOpType


def round_fp32r(a):
    """Round fp32 -> fp32r (1s/8e/11m, RNE), matching walrus fp32_to_fp32r.
    Returns fp32 array whose values are exactly representable in fp32r."""
    u = np.ascontiguousarray(a, dtype=np.float32).view(np.uint32)
    lsb = (u >> np.uint32(12)) & np.uint32(1)
    u2 = u + np.uint32(0x7FF) + lsb          # round-to-nearest-even at bit 12
    u2 &= np.uint32(0xFFFFF000)
    return u2.view(np.float32)


def _patch_tile_drain():
    """This walrus build allows at most one sync wait per TPB_CTRL (Drain)
    instruction; Tile's tail drain carries one wait per active processor.
    Split it into a chain of single-wait drains (same-engine program order
    makes the conjunction equivalent)."""
    if getattr(tile.TileContext, "_drain_split_patch", False):
        return

    def _drain_and_barrier(self, tick_clock, wait_clock):
        drain_inst = self.nc.sync.drain()
        wait_clock.add_sem_waits(
            drain_inst.ins, ScopedClock({None: tick_clock.global_clock})
        )
        waits = (
            list(drain_inst.ins.sync_info.on_wait)
            if drain_inst.ins.has_wait()
            else []
        )
        if len(waits) > 1:
            drain_inst.ins.sync_info.on_wait = waits[:1]
            for i in range(1, len(waits)):
                d2 = self.nc.sync.drain()
                d2.ins.sync_info = _br.SyncInfo(on_wait=waits[i : i + 1], on_update=[])
        self.nc.all_engine_barrier()
        assert self.sems is not None
        popped = self.nc._tile_sem_poison_stack.pop()
        assert popped is self._sem_poison
        self.nc.clear_and_free_semaphores(list(self.sems.allocated().values()))
        self.nc.all_engine_barrier()

    tile.TileContext._drain_and_barrier = _drain_and_barrier

    # Same limit applies to every instruction class (Matmult, DMACopy, ...).
    # Before committing the scheduled instruction stream, shed all but one
    # wait per instruction onto standalone same-engine InstEventSemaphore
    # carriers placed immediately before it (engine program order preserves
    # the conjunction).
    _orig_lower = tile.TileContext._lower_ordered_insts

    def _split_lower(self, ordered):
        for bb_name, insts in ordered.items():
            new = []
            for inst in insts:
                si = inst.sync_info
                if si is not None and len(si.on_wait) > 1:
                    waits = list(si.on_wait)
                    for w in waits[:-1]:
                        ev = mybir.InstEventSemaphore(
                            name=self.nc.get_next_instruction_name(), ins=[], outs=[]
                        )
                        ev.engine = inst.engine
                        ev.sync_info = _br.SyncInfo(on_wait=[w], on_update=[])
                        new.append(ev)
                    inst.sync_info = _br.SyncInfo(
                        on_wait=[waits[-1]], on_update=list(si.on_update)
                    )
                new.append(inst)
            ordered[bb_name] = new
        return _orig_lower(self, ordered)

    tile.TileContext._lower_ordered_insts = _split_lower
    tile.TileContext._drain_split_patch = True


def _install_ntff_hook():
    """Register the axon NTFF profile hook missing from this image's antenv,
    so run_bass_kernel_spmd(trace=True) can report HW exec time."""
    if "antenv.axon_hooks" in sys.modules:
        return
    try:
        import antenv
        from trn_agent_boot.trn_boot import _ntff_profile_via_ctypes

        hook = _ntff_profile_via_ctypes("/opt/axon/libaxon_pjrt.so")
        mod = types.ModuleType("antenv.axon_hooks")
        mod.get_axon_ntff_profile_hook = lambda: hook
        mod.set_axon_ntff_profile_hook = lambda h: None
        sys.modules["antenv.axon_hooks"] = mod
        antenv.axon_hooks = mod
    except Exception:
        pass  # tracing degrades; execution still works


def build_nc(hilo=True):
    """One SPMD NeuronCore program; all 8 cores run it on their own shard."""
    _patch_tile_drain()
    nc = bass.Bass()
    xT = nc.dram_tensor("xT", [T, D, S], F32, kind="ExternalInput")
    whalves = [nc.dram_tensor("wT_hi", [D, C], F32R, kind="ExternalInput")]
    if hilo:
        whalves.append(nc.dram_tensor("wT_lo", [D, C], F32R, kind="ExternalInput"))
    y = nc.dram_tensor("y", [T, S, C], F32, kind="ExternalOutput")
    NH = len(whalves)

    with tile.TileContext(nc) as tc:
        with (
            tc.tile_pool(name="wpool", bufs=1) as wpool,
            tc.tile_pool(name="vpool", bufs=1) as vpool,
            tc.tile_pool(name="xpool", bufs=5) as xpool,
            tc.tile_pool(name="spool", bufs=2) as spool,
            tc.tile_pool(name="opool", bufs=6) as opool,
            tc.tile_pool(name="ppool", bufs=8, space="PSUM") as ppool,
        ):
            # Startup-critical ordering (subtile deps let consumers start on
            # partially-loaded tiles): first column-quarter of x(t=0) loads
            # first, then W (needed by the first matmul), then the rest of x0.
            QS = [(0, 384), (384, 384), (768, 384), (1152, S - 1152)]
            x0 = [xpool.tile([P, S], F32, tag="x", name=f"x0{d}") for d in range(DCH)]
            q0, qn = QS[0]
            for d in range(DCH):
                nc.sync.dma_start(
                    out=x0[d][:, q0 : q0 + qn],
                    in_=xT[0, d * P : (d + 1) * P, q0 : q0 + qn],
                )

            wt = [[None] * DCH for _ in range(NH)]
            for h in range(NH):
                for d in range(DCH):
                    w = wpool.tile([P, C], F32R, tag=f"w{h}{d}", name=f"w{h}{d}")
                    nc.sync.dma_start(out=w[:], in_=whalves[h][d * P : (d + 1) * P, :])
                    wt[h][d] = w

            for q0, qn in QS[1:]:
                for d in range(DCH):
                    nc.sync.dma_start(
                        out=x0[d][:, q0 : q0 + qn],
                        in_=xT[0, d * P : (d + 1) * P, q0 : q0 + qn],
                    )

            v = [None] * DCH
            xnext = x0
            for t in range(T):
                xcur, xnext = xnext, []
                sp = []
                if t == 0:
                    for d in range(DCH):
                        sp.append(
                            spool.tile([P, S], F32R, tag=f"sp{d}", name=f"sp{d}")
                        )
                        v[d] = vpool.tile([P, S], F32, tag=f"v{d}", name=f"v{d}")
                    for q0, qn in QS:
                        for d in range(DCH):
                            xq = xcur[d][:, q0 : q0 + qn]
                            sq = sp[d][:, q0 : q0 + qn]
                            # v' = 0.5*x (exact; matches v + (x-v)/2 with v=0)
                            nc.vector.tensor_scalar(
                                out=xq, in0=xq, scalar1=0.5, scalar2=None,
                                op0=ALU.mult,
                            )
                            nc.vector.tensor_scalar(
                                out=sq, in0=xq, scalar1=1.0, scalar2=None,
                                op0=ALU.is_ge,
                            )
                            nc.vector.scalar_tensor_tensor(
                                out=v[d][:, q0 : q0 + qn], in0=xq, scalar=1.0,
                                in1=xq, op0=ALU.is_lt, op1=ALU.mult,
                            )
                else:
                    for d in range(DCH):
                        xt = xcur[d]
                        # d = x - v, then v' = (d * 0.5) + v -- exact
                        # replication of the reference's rounding order
                        nc.vector.tensor_sub(xt[:], xt[:], v[d][:])
                        nc.vector.scalar_tensor_tensor(
                            out=xt[:], in0=xt[:], scalar=0.5, in1=v[d][:],
                            op0=ALU.mult, op1=ALU.add,
                        )
                        st = spool.tile([P, S], F32R, tag=f"sp{d}", name=f"sp{d}")
                        nc.vector.tensor_scalar(
                            out=st[:], in0=xt[:], scalar1=1.0, scalar2=None,
                            op0=ALU.is_ge,
                        )
                        sp.append(st)
                        if t < T - 1:
                            # v = (v' < 1) * v' (exact reset, spike in {0,1})
                            nc.vector.scalar_tensor_tensor(
                                out=v[d][:], in0=xt[:], scalar=1.0, in1=xt[:],
                                op0=ALU.is_lt, op1=ALU.mult,
                            )

                if t + 1 < T:
                    for d in range(DCH):
                        xt = xpool.tile([P, S], F32, tag="x", name=f"x{t+1}{d}")
                        nc.sync.dma_start(
                            out=xt[:], in_=xT[t + 1, d * P : (d + 1) * P, :]
                        )
                        xnext.append(xt)

                for k in range(SCH):
                    col0 = k * P
                    m = min(P, S - col0)
                    # one PSUM bank per C-half: finer-grained release lets the
                    # next chunk's matmuls start as soon as one bank is copied
                    ot = opool.tile([P, C], F32, tag="out")
                    for ci, (c0, cn) in enumerate(CHALF):
                        ps = ppool.tile([P, 512], F32, tag="ps")
                        for d in range(DCH):
                            lhsT = sp[d][:, col0 : col0 + m]
                            for h in range(NH):
                                nc.tensor.matmul(
                                    ps[:m, :cn],
                                    lhsT,
                                    wt[h][d][:, c0 : c0 + cn],
                                    start=(d == 0 and h == 0),
                                    stop=(d == DCH - 1 and h == NH - 1),
                                )
                        nc.scalar.copy(out=ot[:m, c0 : c0 + cn], in_=ps[:m, :cn])
                    nc.sync.dma_start(out=y[t, col0 : col0 + m, :], in_=ot[:m])
    return nc


_NC_CACHE = {}


def _get_nc(hilo=True):
    key = ("nc", hilo)
    if key not in _NC_CACHE:
        _NC_CACHE[key] = build_nc(hilo)
    return _NC_CACHE[key]


def _make_in_maps(x, W, hilo=True):
    WT = np.ascontiguousarray(W.T)  # [D, C]
    whi = round_fp32r(WT)
    maps_w = {"wT_hi": whi}
    if hilo:
        # residual is exact in fp32 (Sterbenz); round it to fp32r
        maps_w["wT_lo"] = round_fp32r(WT - whi)
    in_maps = []
    for c in range(NCORES):
        xc = x[:, c * BL : (c + 1) * BL].reshape(T, S, D)
        m = {"xT": np.ascontiguousarray(xc.transpose(0, 2, 1))}
        m.update(maps_w)
        in_maps.append(m)
    return in_maps


def kernel(x, W, b):
    from concourse.bass_utils import run_bass_kernel_spmd

    _install_ntff_hook()
    x = np.asarray(x, dtype=np.float32)
    W = np.asarray(W, dtype=np.float32)
    b = np.asarray(b, dtype=np.float32)

    # hilo=False (default): single fp32r pass, rel err ~1.2e-4 (TF32-class
    # matmul precision), ~130us. KERNEL_HILO=1: exact-fp32 hi+lo split
    # (rel err ~2e-7) at ~1.6x the time.
    hilo = os.environ.get("KERNEL_HILO", "0") == "1"
    nc = _get_nc(hilo)
    in_maps = _make_in_maps(x, W, hilo)
    res = run_bass_kernel_spmd(nc, in_maps, list(range(NCORES)))
    y = np.concatenate(
        [res.results[c]["y"].reshape(T, BL, N, C) for c in range(NCORES)], axis=1
    )
    if np.any(b):
        y = y + b[None, None, None, :]
    return np.ascontiguousarray(y, dtype=np.float32)



# revision 1
# speedup vs baseline: 2.3610x; 2.3610x over previous
"""Trainium2 Bass kernel for nn_Decoder_10110353014984.

Computation (see reference): hard-reset LIF over T=4 steps followed by a
linear head:
    v' = v + (x_t - v)/2 ; spike = (v' >= 1) ; v = (1-spike) * v'
    y  = einsum('tbnd,cd->tbnc', spikes, W) + b

The LIF is replicated with the reference's exact fp32 rounding order:
    d = x - v ; h = 0.5*d (exact) ; v' = v + h ; spike = v' >= 1 ;
    v = v' * (v' < 1)

Sharding: data-parallel over batch B=64 -> 8 per NeuronCore. The host
pre-transposes each shard to xT[T, D, S] (d-major) so LIF spike tiles are
directly the matmul stationary operand (no on-chip transposes), and
pre-transposes W to W^T[D, C] for the moving operand.

Matmul dtype: float32r (TF32-like, 1s/8e/11m, ~4x the fp32 matmul rate).
Spikes are {0,1} (exact in fp32r). Default: W rounded to fp32r on the host,
single pass -> rel err ~1.2e-4 (TF32-class), ~130us/run. KERNEL_HILO=1
splits W into fp32r-exact hi+lo parts (hi+lo == W exactly: 12+12 mantissa
bits) accumulated in one PSUM group -> fp32-exact result (~2e-7) at ~1.6x
the time. Bias is applied host-side (zeros in the spec).
"""

import os
import sys
import types

sys.path.insert(0, "/opt/trn_rl_repo")

import numpy as np

import concourse.bass as bass
import concourse.mybir as mybir
import concourse.tile as tile
from concourse.vector_clock import ScopedClock
import bass_rust as _br

T, B, N, D, C = 4, 64, 196, 512, 1000
NCORES = 8
BL = B // NCORES          # 8 batches per core
S = BL * N                # 1568 samples per timestep per core
P = 128                   # partition width
DCH = D // P              # 4 contraction tiles
SCH = (S + P - 1) // P    # 13 sample chunks (last has 32 rows)
CHALF = [(0, 500), (500, 500)]  # C split across two PSUM banks

F32 = mybir.dt.float32
F32R = mybir.dt.float32r
ALU = mybir.AluOpType


def round_fp32r(a):
    """Round fp32 -> fp32r (1s/8e/11m, RNE), matching walrus fp32_to_fp32r.
    Returns fp32 array whose values are exactly representable in fp32r."""
    u = np.ascontiguousarray(a, dtype=np.float32).view(np.uint32)
    lsb = (u >> np.uint32(12)) & np.uint32(1)
    u2 = u + np.uint32(0x7FF) + lsb          # round-to-nearest-even at bit 12
    u2 &= np.uint32(0xFFFFF000)
    return u2.view(np.float32)


def _patch_tile_drain():
    """This walrus build allows at most one sync wait per TPB_CTRL (Drain)
    instruction; Tile's tail drain carries one wait per active processor.
    Split it into a chain of single-wait drains (same-engine program order
    makes the conjunction equivalent)."""
    if getattr(tile.TileContext, "_drain_split_patch", False):
        return

    def _drain_and_barrier(self, tick_clock, wait_clock):
        drain_inst = self.nc.sync.drain()
        wait_clock.add_sem_waits(
            drain_inst.ins, ScopedClock({None: tick_clock.global_clock})
        )
        waits = (
            list(drain_inst.ins.sync_info.on_wait)
            if drain_inst.ins.has_wait()
            else []
        )
        if len(waits) > 1:
            drain_inst.ins.sync_info.on_wait = waits[:1]
            for i in range(1, len(waits)):
                d2 = self.nc.sync.drain()
                d2.ins.sync_info = _br.SyncInfo(on_wait=waits[i : i + 1], on_update=[])
        self.nc.all_engine_barrier()
        assert self.sems is not None
        popped = self.nc._tile_sem_poison_stack.pop()
        assert popped is self._sem_poison
        self.nc.clear_and_free_semaphores(list(self.sems.allocated().values()))
        self.nc.all_engine_barrier()

    tile.TileContext._drain_and_barrier = _drain_and_barrier

    # Same limit applies to every instruction class (Matmult, DMACopy, ...).
    # Before committing the scheduled instruction stream, shed all but one
    # wait per instruction onto standalone same-engine InstEventSemaphore
    # carriers placed immediately before it (engine program order preserves
    # the conjunction).
    _orig_lower = tile.TileContext._lower_ordered_insts

    def _split_lower(self, ordered):
        for bb_name, insts in ordered.items():
            new = []
            for inst in insts:
                si = inst.sync_info
                if si is not None and len(si.on_wait) > 1:
                    waits = list(si.on_wait)
                    for w in waits[:-1]:
                        ev = mybir.InstEventSemaphore(
                            name=self.nc.get_next_instruction_name(), ins=[], outs=[]
                        )
                        ev.engine = inst.engine
                        ev.sync_info = _br.SyncInfo(on_wait=[w], on_update=[])
                        new.append(ev)
                    inst.sync_info = _br.SyncInfo(
                        on_wait=[waits[-1]], on_update=list(si.on_update)
                    )
                new.append(inst)
            ordered[bb_name] = new
        return _orig_lower(self, ordered)

    tile.TileContext._lower_ordered_insts = _split_lower
    tile.TileContext._drain_split_patch = True


def _install_ntff_hook():
    """Register the axon NTFF profile hook missing from this image's antenv,
    so run_bass_kernel_spmd(trace=True) can report HW exec time."""
    if "antenv.axon_hooks" in sys.modules:
        return
    try:
        import antenv
        from trn_agent_boot.trn_boot import _ntff_profile_via_ctypes

        hook = _ntff_profile_via_ctypes("/opt/axon/libaxon_pjrt.so")
        mod = types.ModuleType("antenv.axon_hooks")
        mod.get_axon_ntff_profile_hook = lambda: hook
        mod.set_axon_ntff_profile_hook = lambda h: None
        sys.modules["antenv.axon_hooks"] = mod
        antenv.axon_hooks = mod
    except Exception:
        pass  # tracing degrades; execution still works


def build_nc(hilo=True):
    """One SPMD NeuronCore program; all 8 cores run it on their own shard."""
    _patch_tile_drain()
    nc = bass.Bass()
    xT = nc.dram_tensor("xT", [T, D, S], F32, kind="ExternalInput")
    whalves = [nc.dram_tensor("wT_hi", [D, C], F32R, kind="ExternalInput")]
    if hilo:
        whalves.append(nc.dram_tensor("wT_lo", [D, C], F32R, kind="ExternalInput"))
    y = nc.dram_tensor("y", [T, S, C], F32, kind="ExternalOutput")
    NH = len(whalves)

    with tile.TileContext(nc) as tc:
        with (
            tc.tile_pool(name="wpool", bufs=1) as wpool,
            tc.tile_pool(name="vpool", bufs=1) as vpool,
            tc.tile_pool(name="xpool", bufs=5) as xpool,
            tc.tile_pool(name="spool", bufs=2) as spool,
            tc.tile_pool(name="opool", bufs=6) as opool,
            tc.tile_pool(name="ppool", bufs=8, space="PSUM") as ppool,
        ):
            # Startup-critical ordering (subtile deps let consumers start on
            # partially-loaded tiles): first column-quarter of x(t=0) loads
            # first, then W (needed by the first matmul), then the rest of x0.
            QS = [(0, 384), (384, 384), (768, 384), (1152, S - 1152)]
            x0 = [xpool.tile([P, S], F32, tag="x", name=f"x0{d}") for d in range(DCH)]
            q0, qn = QS[0]
            for d in range(DCH):
                nc.sync.dma_start(
                    out=x0[d][:, q0 : q0 + qn],
                    in_=xT[0, d * P : (d + 1) * P, q0 : q0 + qn],
                )

            wt = [[None] * DCH for _ in range(NH)]
            for h in range(NH):
                for d in range(DCH):
                    w = wpool.tile([P, C], F32R, tag=f"w{h}{d}", name=f"w{h}{d}")
                    nc.sync.dma_start(out=w[:], in_=whalves[h][d * P : (d + 1) * P, :])
                    wt[h][d] = w

            for q0, qn in QS[1:]:
                for d in range(DCH):
                    nc.sync.dma_start(
                        out=x0[d][:, q0 : q0 + qn],
                        in_=xT[0, d * P : (d + 1) * P, q0 : q0 + qn],
                    )

            v = [None] * DCH
            xnext = x0
            for t in range(T):
                xcur, xnext = xnext, []
                sp = []
                if t == 0:
                    for d in range(DCH):
                        sp.append(
                            spool.tile([P, S], F32R, tag=f"sp{d}", name=f"sp{d}")
                        )
                        v[d] = vpool.tile([P, S], F32, tag=f"v{d}", name=f"v{d}")
                    for q0, qn in QS:
                        for d in range(DCH):
                            xq = xcur[d][:, q0 : q0 + qn]
                            sq = sp[d][:, q0 : q0 + qn]
                            # v' = 0.5*x (exact; matches v + (x-v)/2 with v=0)
                            nc.vector.tensor_scalar(
                                out=xq, in0=xq, scalar1=0.5, scalar2=None,
                                op0=ALU.mult,
                            )
                            nc.vector.tensor_scalar(
                                out=sq, in0=xq, scalar1=1.0, scalar2=None,
                                op0=ALU.is_ge,
                            )
                            nc.vector.scalar_tensor_tensor(
                                out=v[d][:, q0 : q0 + qn], in0=xq, scalar=1.0,
                                in1=xq, op0=ALU.is_lt, op1=ALU.mult,
                            )
                else:
                    for d in range(DCH):
                        xt = xcur[d]
                        # d = x - v, then v' = (d * 0.5) + v -- exact
                        # replication of the reference's rounding order
                        nc.vector.tensor_sub(xt[:], xt[:], v[d][:])
                        nc.vector.scalar_tensor_tensor(
                            out=xt[:], in0=xt[:], scalar=0.5, in1=v[d][:],
                            op0=ALU.mult, op1=ALU.add,
                        )
                        st = spool.tile([P, S], F32R, tag=f"sp{d}", name=f"sp{d}")
                        nc.vector.tensor_scalar(
                            out=st[:], in0=xt[:], scalar1=1.0, scalar2=None,
                            op0=ALU.is_ge,
                        )
                        sp.append(st)
                        if t < T - 1:
                            # v = (v' < 1) * v' (exact reset, spike in {0,1})
                            nc.vector.scalar_tensor_tensor(
                                out=v[d][:], in0=xt[:], scalar=1.0, in1=xt[:],
                                op0=ALU.is_lt, op1=ALU.mult,
                            )

                if t + 1 < T:
                    for d in range(DCH):
                        xt = xpool.tile([P, S], F32, tag="x", name=f"x{t+1}{d}")
                        nc.sync.dma_start(
                            out=xt[:], in_=xT[t + 1, d * P : (d + 1) * P, :]
                        )
                        xnext.append(xt)

                for k in range(SCH):
                    col0 = k * P
                    m = min(P, S - col0)
                    # one PSUM bank per C-half: finer-grained release lets the
                    # next chunk's matmuls start as soon as one bank is copied
                    ot = opool.tile([P, C], F32, tag="out")
                    for ci, (c0, cn) in enumerate(CHALF):
                        ps = ppool.tile([P, 512], F32, tag="ps")
                        for d in range(DCH):
                            lhsT = sp[d][:, col0 : col0 + m]
                            for h in range(NH):
                                nc.tensor.matmul(
                                    ps[:m, :cn],
                                    lhsT,
                                    wt[h][d][:, c0 : c0 + cn],
                                    start=(d == 0 and h == 0),
                                    stop=(d == DCH - 1 and h == NH - 1),
                                )
                        nc.scalar.copy(out=ot[:m, c0 : c0 + cn], in_=ps[:m, :cn])
                    nc.sync.dma_start(out=y[t, col0 : col0 + m, :], in_=ot[:m])
    return nc


_NC_CACHE = {}


def _get_nc(hilo=True):
    key = ("nc", hilo)
    if key not in _NC_CACHE:
        _NC_CACHE[key] = build_nc(hilo)
    return _NC_CACHE[key]


def _make_in_maps(x, W, hilo=True):
    WT = np.ascontiguousarray(W.T)  # [D, C]
    whi = round_fp32r(WT)
    maps_w = {"wT_hi": whi}
    if hilo:
        # residual is exact in fp32 (Sterbenz); round it to fp32r
        maps_w["wT_lo"] = round_fp32r(WT - whi)
    in_maps = []
    for c in range(NCORES):
        xc = x[:, c * BL : (c + 1) * BL].reshape(T, S, D)
        m = {"xT": np.ascontiguousarray(xc.transpose(0, 2, 1))}
        m.update(maps_w)
        in_maps.append(m)
    return in_maps


def kernel(x, W, b):
    from concourse.bass_utils import run_bass_kernel_spmd

    _install_ntff_hook()
    x = np.asarray(x, dtype=np.float32)
    W = np.asarray(W, dtype=np.float32)
    b = np.asarray(b, dtype=np.float32)

    # hilo=False (default): single fp32r pass, rel err ~1.2e-4 (TF32-class
    # matmul precision), ~130us. KERNEL_HILO=1: exact-fp32 hi+lo split
    # (rel err ~2e-7) at ~1.6x the time.
    hilo = os.environ.get("KERNEL_HILO", "0") == "1"
    nc = _get_nc(hilo)
    in_maps = _make_in_maps(x, W, hilo)
    res = run_bass_kernel_spmd(nc, in_maps, list(range(NCORES)))
    y = np.concatenate(
        [res.results[c]["y"].reshape(T, BL, N, C) for c in range(NCORES)], axis=1
    )
    if np.any(b):
        y = y + b[None, None, None, :]
    return np.ascontiguousarray(y, dtype=np.float32)

